# revision 1
# baseline (speedup 1.0000x reference)
"""Calibrated cross-entropy 2D (histogram binning) — Trainium2 Bass kernel.

Problem: nn_CalibratedCE2d_88493506167215
  predict    [8, 21, 513, 513] f32   (NCHW logits)
  target     [8, 513, 513]     int   (class ids)
  confidence [2105352]         f32
  accuracies [15]              f32
  n_bin      15

  loss = -sum_i w_i * logp_target_i / size
  where w_i = coeff[bin(confidence_i)] if selected else 0,
        coeff_b = acc_b*10 - (1-acc_b)*50 (only coeff>0 bins selected),
        size = number of selected pixels.

Sharding: data-parallel over the batch axis — one image (n) per NeuronCore,
8 cores.  Per-core device program (pixel-major [128, F] tiles):
  for each class c in 0..20:
      load plane slice x_c, e_c = exp(x_c)                 (ACT, bf16 out)
      masked_c = (tgt == c) * e_c                          (DVE fused stt)
      PSUM A += I @ e_c ; PSUM B += I @ masked_c           (PE identity matmuls)
  A = sum_c exp(x_c) per pixel, B = exp(x_target) per pixel
  logp_t = ln(B) - ln(A)
  out partials: sum_f w*ln(B), sum_f w*ln(A)               (DVE stt + accum)
Host: per-pixel weights w from confidence (identical f32 arithmetic as the
reference), 8-way partial-sum combine, final divide.  The last pixel of each
image (263169 = 128*2056 + 1 does not tile evenly) is folded in on the host.
"""

import numpy as np
import ml_dtypes
from contextlib import ExitStack

N_IMG, C, H, W = 8, 21, 513, 513
PX = H * W                    # 263169 pixels per image
FD = 2056                     # tile free dim (2048 main grid + 8 tail cols)
MFD = 2048                    # main grid columns -> PSUM chains (4 banks each)
MAIN = 128 * MFD              # 262144 pixels in the main grid
LEFT = MAIN + 128 * 8         # 263168; the final pixel is handled on the host
N_TOTAL_BINS = 15

_NC_CACHE: dict = {}


def _build_program():
    import concourse.bass as bass
    import concourse.bacc as bacc
    import concourse.tile as tile
    from concourse import mybir

    f32 = mybir.dt.float32
    bf16 = mybir.dt.bfloat16
    Exp = mybir.ActivationFunctionType.Exp
    Ln = mybir.ActivationFunctionType.Ln
    is_equal = mybir.AluOpType.is_equal
    mult = mybir.AluOpType.mult
    bypass = mybir.AluOpType.bypass

    nc = bacc.Bacc(
        "TRN2",
        target_bir_lowering=False,
        debug=False,
        enable_asserts=False,
        num_devices=N_IMG,
    )
    x_d = nc.dram_tensor("x", [C, PX], f32, kind="ExternalInput")
    tgt_d = nc.dram_tensor("tgt", [PX], bf16, kind="ExternalInput")
    w_d = nc.dram_tensor("w", [PX], f32, kind="ExternalInput")
    id_d = nc.dram_tensor("ident", [128, 128], bf16, kind="ExternalInput")
    # host-packed tail sidecar: pixels MAIN..LEFT as [128, 21*8] / [128, 8]
    xt_d = nc.dram_tensor("xt", [128, C * 8], f32, kind="ExternalInput")
    mkt_d = nc.dram_tensor("mkt", [128, C * 8], bf16, kind="ExternalInput")
    w8_d = nc.dram_tensor("w8", [128, 8], f32, kind="ExternalInput")
    out_d = nc.dram_tensor("out", [128, 10], f32, kind="ExternalOutput")

    x = x_d.ap()
    tgt = tgt_d.ap()
    w = w_d.ap()

    with tile.TileContext(nc) as tc, ExitStack() as ctx:
        const_pool = ctx.enter_context(tc.tile_pool(name="const", bufs=1))
        xpool = ctx.enter_context(tc.tile_pool(name="xp", bufs=8))
        epool = ctx.enter_context(tc.tile_pool(name="ep", bufs=8))
        kpool = ctx.enter_context(tc.tile_pool(name="kp", bufs=3))
        mpool = ctx.enter_context(tc.tile_pool(name="mp", bufs=8))
        postpool = ctx.enter_context(tc.tile_pool(name="post", bufs=1))
        psum = ctx.enter_context(tc.tile_pool(name="ps", bufs=1, space="PSUM"))

        zb = const_pool.tile([128, 1], f32, tag="zb", name="zb")
        nc.vector.memset(zb[:], 0.0)
        ob = const_pool.tile([128, 1], f32, tag="ob", name="ob")
        nc.vector.memset(ob[:], 1.0)
        # dummy activations: hoist the ACT table loads to kernel start so
        # they overlap the DMA ramp instead of gating the first/last real op
        dum = const_pool.tile([128, 2], f32, tag="dum", name="dum")
        nc.scalar.activation(dum[:, 0:1], zb[:], Ln, bias=ob[:, 0:1])
        nc.scalar.activation(dum[:, 1:2], zb[:], Exp, bias=zb[:, 0:1])

        tgt_m = const_pool.tile([128, MFD], bf16, tag="tgtm", name="tgt_m")
        w_m = const_pool.tile([128, MFD], f32, tag="wm", name="w_m")
        idt = const_pool.tile([128, 128], bf16, tag="idt", name="idt")
        xt = const_pool.tile([128, C * 8], f32, tag="xt", name="xt")
        mkt = const_pool.tile([128, C * 8], bf16, tag="mkt", name="mkt")
        w8 = const_pool.tile([128, 8], f32, tag="w8", name="w8")

        # A = sum_c exp(x_c), B = exp(x_target): PE psum chains over the main
        # 2048 columns; the 1024-px tail sidecar reduces on DVE.
        A = psum.tile([128, MFD], f32, tag="A", name="A")
        B = psum.tile([128, MFD], f32, tag="B", name="B")

        def load_x(c):
            t = xpool.tile([128, MFD], f32, tag="xm", name=f"xm{c}")
            nc.sync.dma_start(
                t[:], x[c : c + 1, 0:MAIN].rearrange("o (p f) -> (o p) f", p=128)
            )
            return t

        acc = postpool.tile([128, 10], f32, tag="acc", name="acc")
        nc.vector.memset(acc[:], 0.0)

        def emit_tail_sidecar():
            # 1024-px tail: one exp + mask-mul + class-axis reduces + post.
            # No dependency on the psum chains — emitted mid-loop so it
            # fills DMA-wait bubbles instead of serializing at the end.
            et_all = const_pool.tile([128, C * 8], bf16, tag="eta", name="et_all")
            nc.scalar.activation(et_all[:], xt[:], Exp, bias=zb[:, 0:1])
            mt_all = const_pool.tile([128, C * 8], bf16, tag="mta", name="mt_all")
            nc.vector.tensor_tensor(mt_all[:], mkt[:], et_all[:], op=mult)
            At = const_pool.tile([128, 8], f32, tag="At", name="At")
            Bt = const_pool.tile([128, 8], f32, tag="Bt", name="Bt")
            nc.vector.tensor_reduce(
                At[:], et_all[:].rearrange("p (c j) -> p j c", c=C),
                axis=mybir.AxisListType.X, op=mybir.AluOpType.add,
            )
            nc.vector.tensor_reduce(
                Bt[:], mt_all[:].rearrange("p (c j) -> p j c", c=C),
                axis=mybir.AxisListType.X, op=mybir.AluOpType.add,
            )
            lbt = const_pool.tile([128, 8], f32, tag="lbt", name="lbt")
            lat = const_pool.tile([128, 8], f32, tag="lat", name="lat")
            scrt = const_pool.tile([128, 8], f32, tag="scrt", name="scrt")
            nc.scalar.activation(lbt[:], Bt[:], Ln, bias=zb[:, 0:1])
            nc.scalar.activation(lat[:], At[:], Ln, bias=zb[:, 0:1])
            nc.vector.scalar_tensor_tensor(
                scrt[:], lbt[:], 0.0, w8[:],
                op0=bypass, op1=mult, accum_out=acc[:, 8:9],
            )
            nc.vector.scalar_tensor_tensor(
                scrt[:], lat[:], 0.0, w8[:],
                op0=bypass, op1=mult, accum_out=acc[:, 9:10],
            )

        xms = {0: load_x(0)}
        for c in range(C):
            xm = xms.pop(c)
            if c == 0:
                nc.sync.dma_start(
                    tgt_m[:], tgt[0:MAIN].rearrange("(p f) -> p f", p=128)
                )
                nc.sync.dma_start(idt[:], id_d.ap())
            if c + 1 < C:
                xms[c + 1] = load_x(c + 1)
            if c == 2:
                nc.sync.dma_start(xt[:], xt_d.ap())
                nc.sync.dma_start(mkt[:], mkt_d.ap())
                nc.sync.dma_start(w8[:], w8_d.ap())
            if c == 4:
                nc.sync.dma_start(
                    w_m[:], w[0:MAIN].rearrange("(p f) -> p f", p=128)
                )
            em = epool.tile([128, MFD], bf16, tag="em", name=f"em{c}")
            nc.scalar.activation(em[:], xm[:], Exp, bias=zb[:, 0:1])
            mk = kpool.tile([128, MFD], bf16, tag="mk", name=f"mk{c}")
            nc.vector.tensor_scalar(mk[:], tgt_m[:], float(c), None, op0=is_equal)
            mm = mpool.tile([128, MFD], bf16, tag="mm", name=f"mm{c}")
            nc.vector.tensor_tensor(mm[:], mk[:], em[:], op=mult)
            for j in range(MFD // 512):
                sl = slice(j * 512, (j + 1) * 512)
                nc.tensor.matmul(
                    A[:, sl], idt[:], em[:, sl], start=(c == 0), stop=(c == C - 1)
                )
                nc.tensor.matmul(
                    B[:, sl], idt[:], mm[:, sl], start=(c == 0), stop=(c == C - 1)
                )
            if c == 5:
                emit_tail_sidecar()

        # ---- post: logp_t = ln(B) - ln(A); accumulate w-weighted sums.
        # Column halves pipeline ACT(ln) with DVE(weighted reduce).
        lb = postpool.tile([128, MFD], f32, tag="lb", name="lb")
        la = postpool.tile([128, MFD], f32, tag="la", name="la")
        scr = postpool.tile([128, MFD], f32, tag="scr", name="scr")
        HH = MFD // 2
        for h in range(2):
            sl = slice(h * HH, (h + 1) * HH)
            nc.scalar.activation(lb[:, sl], B[:, sl], Ln, bias=zb[:, 0:1])
            nc.vector.scalar_tensor_tensor(
                scr[:, sl], lb[:, sl], 0.0, w_m[:, sl],
                op0=bypass, op1=mult, accum_out=acc[:, 4 * h : 4 * h + 1],
            )
            nc.scalar.activation(la[:, sl], A[:, sl], Ln, bias=zb[:, 0:1])
            nc.vector.scalar_tensor_tensor(
                scr[:, sl], la[:, sl], 0.0, w_m[:, sl],
                op0=bypass, op1=mult, accum_out=acc[:, 4 * h + 1 : 4 * h + 2],
            )
        nc.sync.dma_start(out_d.ap(), acc[:])

    nc.compile()
    return nc


def _get_nc():
    if "nc" not in _NC_CACHE:
        _NC_CACHE["nc"] = _build_program()
    return _NC_CACHE["nc"]


def _pixel_weights(conf: np.ndarray, accuracies: np.ndarray, n_bin: int):
    """Per-pixel weights, f32 arithmetic identical to the reference."""
    acc = np.asarray(accuracies, dtype=np.float32)[:n_bin]
    coeff = acc * np.float32(10.0) - (np.float32(1.0) - acc) * np.float32(50.0)
    wtab = np.where(coeff > np.float32(0.0), coeff, np.float32(0.0)).astype(np.float32)
    # table16[k] for k = ceil(conf*15) in 0..15; k=0 (conf==0) -> invalid -> 0
    table16 = np.concatenate([[np.float32(0.0)], wtab]).astype(np.float32)
    t15 = conf * np.float32(N_TOTAL_BINS)          # same f32 product as reference
    k16 = np.ceil(t15).astype(np.int32)
    k16 = np.clip(k16, 0, n_bin)
    wfull = table16[k16]
    valid = (conf > np.float32(0.0)) & (conf <= np.float32(1.0))
    wfull = np.where(valid, wfull, np.float32(0.0)).astype(np.float32)
    return wfull


def _prepare(predict, target, confidence, accuracies, n_bin):
    predict = np.ascontiguousarray(np.asarray(predict, dtype=np.float32))
    target = np.asarray(target)
    conf = np.asarray(confidence, dtype=np.float32)
    accuracies = np.asarray(accuracies, dtype=np.float32)
    n_bin = int(n_bin)
    assert predict.shape == (N_IMG, C, H, W) and n_bin == N_TOTAL_BINS

    wfull = _pixel_weights(conf, accuracies, n_bin)
    size = float(np.count_nonzero(wfull))

    xs = predict.reshape(N_IMG, C, PX)
    tg = target.reshape(N_IMG, PX).astype(np.int64)
    wf = wfull.reshape(N_IMG, PX)
    ident = np.eye(128, dtype=ml_dtypes.bfloat16)

    in_maps = []
    for n in range(N_IMG):
        # tail sidecar: pixels MAIN..LEFT as [128, 8], classes side by side
        xt = np.ascontiguousarray(
            xs[n][:, MAIN:LEFT].reshape(C, 128, 8).transpose(1, 0, 2).reshape(128, C * 8)
        )
        tail_t = tg[n][MAIN:LEFT].reshape(128, 8)
        onehot = (tail_t[None, :, :] == np.arange(C)[:, None, None])
        mkt = np.ascontiguousarray(
            onehot.transpose(1, 0, 2).reshape(128, C * 8)
        ).astype(ml_dtypes.bfloat16)
        w8 = np.ascontiguousarray(wf[n][MAIN:LEFT].reshape(128, 8))
        in_maps.append(
            {
                "x": xs[n],
                "tgt": tg[n].astype(ml_dtypes.bfloat16),
                "w": wf[n],
                "ident": ident,
                "xt": xt,
                "mkt": mkt,
                "w8": w8,
            }
        )
    return xs, tg, wf, size, in_maps


def _combine(res_list, xs, tg, wf, size) -> np.ndarray:
    S = 0.0
    for n in range(N_IMG):
        o = np.asarray(res_list[n]["out"], dtype=np.float64)
        # written accumulator columns: (w*lnB, w*lnA) pairs at 0,4,8
        S += sum(o[:, j].sum() - o[:, j + 1].sum() for j in (0, 4, 8))

    # host-side leftover pixels (one per image: index LEFT..PX-1)
    for n in range(N_IMG):
        for p in range(LEFT, PX):
            xv = xs[n][:, p].astype(np.float64)
            m = xv.max()
            lse = np.log(np.exp(xv - m).sum()) + m
            xt = xv[tg[n][p]]
            S += float(wf[n][p]) * (xt - lse)

    loss = np.float32(-(S / size))
    return np.asarray(loss, dtype=np.float32)


def run_device(in_maps, trace=False, **kwargs):
    from concourse.bass_utils import run_bass_kernel_spmd

    nc = _get_nc()
    return run_bass_kernel_spmd(
        nc, in_maps, core_ids=list(range(N_IMG)), trace=trace, **kwargs
    )


def kernel(predict, target, confidence, accuracies, n_bin) -> np.ndarray:
    xs, tg, wf, size, in_maps = _prepare(predict, target, confidence, accuracies, n_bin)
    res = run_device(in_maps)
    return _combine(res.results, xs, tg, wf, size)



# revision 5
# speedup vs baseline: 3.2442x; 3.2442x over previous
"""Calibrated cross-entropy 2D (histogram binning) — Trainium2 Bass kernel.

Problem: nn_CalibratedCE2d_88493506167215
  predict    [8, 21, 513, 513] f32   (NCHW logits)
  target     [8, 513, 513]     int   (class ids)
  confidence [2105352]         f32
  accuracies [15]              f32
  n_bin      15

  loss = -sum_i w_i * logp_target_i / size
  where w_i = coeff[bin(confidence_i)] if selected else 0,
        coeff_b = acc_b*10 - (1-acc_b)*50 (only coeff>0 bins selected),
        size = number of selected pixels.

Key observation: pixels with w_i == 0 (w is a host-side function of
confidence/accuracies only, as in the baseline) contribute nothing to the
loss numerator or denominator, so only the w>0 pixels (~20% for the staged
inputs) need their log-softmax evaluated.  The host packs the alive pixels
of each image into a [128, C*SC] grid, grouped by target class so the
target-gather becomes a contiguous slice, and uploads logits in fp16.

Sharding: one image per NeuronCore, 8 cores (data-parallel over batch).

Per-core device program (grid [128 partitions, FD=C*SC pixel columns]):
  x tile [128, C*FD]: plane c at [:, c*FD:(c+1)*FD] = class-c logits of all
  grid pixels; class-c's own pixels sit at grid columns [c*SC,(c+1)*SC).
    exp chunks (ACT)  : em = exp(x) over groups of classes
    A-chain    (PE)   : A[p,f] += I @ em_c  (PSUM, per-pixel sum over classes)
    S2 partial (DVE)  : per class c, ttr( x_c[:, own seg] * w[:, seg] ) -> acc
    lnA        (ACT)  : la = ln(A)
    S1 partial (DVE)  : ttr( la * w ) -> acc
Host: build w/grid, 8-way combine, loss = -(S2-S1)/size.
"""

import numpy as np
from contextlib import ExitStack

N_IMG, C, H, W = 8, 21, 513, 513
PX = H * W                    # 263169 pixels per image
N_TOTAL_BINS = 15
SC_DEFAULT = 24               # grid columns per class (capacity 128*SC pixels)

_NC_CACHE: dict = {}


def _build_program(SC):
    import concourse.bass as bass
    import concourse.bacc as bacc
    import concourse.tile as tile
    from concourse import mybir

    f32 = mybir.dt.float32
    fp16 = mybir.dt.float16
    Exp = mybir.ActivationFunctionType.Exp
    Ln = mybir.ActivationFunctionType.Ln
    mult = mybir.AluOpType.mult
    bypass = mybir.AluOpType.bypass

    FD = C * SC               # pixel-grid columns
    CHG = 3                   # classes per exp chunk
    assert C % CHG == 0
    NCH = C // CHG

    nc = bacc.Bacc(
        "TRN2",
        target_bir_lowering=False,
        debug=False,
        enable_asserts=False,
        num_devices=N_IMG,
    )
    x_d = nc.dram_tensor("x", [128, C * FD], fp16, kind="ExternalInput")
    w_d = nc.dram_tensor("w", [128, FD], fp16, kind="ExternalInput")
    id_d = nc.dram_tensor("ident", [128, 128], fp16, kind="ExternalInput")
    out_d = nc.dram_tensor("out", [128, C + 1], f32, kind="ExternalOutput")

    with tile.TileContext(nc) as tc, ExitStack() as ctx:
        const_pool = ctx.enter_context(tc.tile_pool(name="const", bufs=1))
        empool = ctx.enter_context(tc.tile_pool(name="ep", bufs=3))
        scrpool = ctx.enter_context(tc.tile_pool(name="scp", bufs=2))
        psum = ctx.enter_context(tc.tile_pool(name="ps", bufs=1, space="PSUM"))

        zb = const_pool.tile([128, 1], f32, tag="zb", name="zb")
        nc.vector.memset(zb[:], 0.0)
        ob = const_pool.tile([128, 1], f32, tag="ob", name="ob")
        nc.vector.memset(ob[:], 1.0)
        # hoist ACT table loads to kernel start (overlap the DMA ramp)
        dum = const_pool.tile([128, 2], f32, tag="dum", name="dum")
        nc.scalar.activation(dum[:, 0:1], zb[:], Ln, bias=ob[:, 0:1])
        nc.scalar.activation(dum[:, 1:2], zb[:], Exp, bias=zb[:, 0:1])

        xall = const_pool.tile([128, C * FD], fp16, tag="xall", name="xall")
        w_t = const_pool.tile([128, FD], fp16, tag="wt", name="w_t")
        idt = const_pool.tile([128, 128], fp16, tag="idt", name="idt")
        acc = const_pool.tile([128, C + 1], f32, tag="acc", name="acc")

        A = psum.tile([128, FD], f32, tag="A", name="A")

        x = x_d.ap()
        # prefetch everything; per-chunk DMAs let exp start early
        nc.sync.dma_start(w_t[:], w_d.ap())
        nc.sync.dma_start(idt[:], id_d.ap())
        for k in range(NCH):
            sl = slice(k * CHG * FD, (k + 1) * CHG * FD)
            nc.sync.dma_start(xall[:, sl], x[:, sl])

        nslice = (FD + 511) // 512
        for k in range(NCH):
            em = empool.tile([128, CHG * FD], fp16, tag="em", name=f"em{k}")
            nc.scalar.activation(
                em[:], xall[:, k * CHG * FD : (k + 1) * CHG * FD], Exp,
                bias=zb[:, 0:1],
            )
            for j in range(CHG):
                c = k * CHG + j
                for s in range(nslice):
                    sl = slice(s * 512, min(FD, (s + 1) * 512))
                    nc.tensor.matmul(
                        A[:, sl], idt[:], em[:, j * FD + sl.start : j * FD + sl.stop],
                        start=(c == 0), stop=(c == C - 1),
                    )
                # S2 partials: class-c pixels live at grid cols [c*SC,(c+1)*SC)
                scr = scrpool.tile([128, SC], f32, tag="scr", name=f"scr{c}")
                nc.vector.scalar_tensor_tensor(
                    scr[:],
                    xall[:, c * FD + c * SC : c * FD + (c + 1) * SC],
                    0.0,
                    w_t[:, c * SC : (c + 1) * SC],
                    op0=bypass, op1=mult,
                    accum_out=acc[:, 1 + c : 2 + c],
                )

        la = const_pool.tile([128, FD], fp16, tag="la", name="la")
        nc.scalar.activation(la[:], A[:], Ln, bias=zb[:, 0:1])
        scr2 = const_pool.tile([128, FD], f32, tag="scr2", name="scr2")
        nc.vector.scalar_tensor_tensor(
            scr2[:], la[:], 0.0, w_t[:], op0=bypass, op1=mult,
            accum_out=acc[:, 0:1],
        )
        nc.sync.dma_start(out_d.ap(), acc[:])

    nc.compile()
    return nc


def _get_nc(SC=SC_DEFAULT):
    if SC not in _NC_CACHE:
        _NC_CACHE[SC] = _build_program(SC)
    return _NC_CACHE[SC]


def _pixel_weights(conf: np.ndarray, accuracies: np.ndarray, n_bin: int):
    """Per-pixel weights, f32 arithmetic identical to the reference."""
    acc = np.asarray(accuracies, dtype=np.float32)[:n_bin]
    coeff = acc * np.float32(10.0) - (np.float32(1.0) - acc) * np.float32(50.0)
    wtab = np.where(coeff > np.float32(0.0), coeff, np.float32(0.0)).astype(np.float32)
    # table16[k] for k = ceil(conf*15) in 0..15; k=0 (conf==0) -> invalid -> 0
    table16 = np.concatenate([[np.float32(0.0)], wtab]).astype(np.float32)
    t15 = conf * np.float32(N_TOTAL_BINS)          # same f32 product as reference
    k16 = np.ceil(t15).astype(np.int32)
    k16 = np.clip(k16, 0, n_bin)
    wfull = table16[k16]
    valid = (conf > np.float32(0.0)) & (conf <= np.float32(1.0))
    wfull = np.where(valid, wfull, np.float32(0.0)).astype(np.float32)
    return wfull


def _prepare(predict, target, confidence, accuracies, n_bin):
    predict = np.ascontiguousarray(np.asarray(predict, dtype=np.float32))
    target = np.asarray(target)
    conf = np.asarray(confidence, dtype=np.float32)
    accuracies = np.asarray(accuracies, dtype=np.float32)
    n_bin = int(n_bin)
    assert predict.shape == (N_IMG, C, H, W) and n_bin == N_TOTAL_BINS

    wfull = _pixel_weights(conf, accuracies, n_bin)
    size = float(np.count_nonzero(wfull))

    xs = predict.reshape(N_IMG, C, PX)
    tg = target.reshape(N_IMG, PX).astype(np.int32)
    wf = wfull.reshape(N_IMG, PX)

    # capacity needed: max alive-per-class count across cores
    maxcnt = 1
    percore = []
    for n in range(N_IMG):
        alive = np.nonzero(wf[n])[0].astype(np.int32)
        ta = tg[n][alive]
        order = np.argsort(ta, kind="stable")
        ids = alive[order]
        cnts = np.bincount(ta, minlength=C)
        maxcnt = max(maxcnt, int(cnts.max()))
        percore.append((ids, cnts))
    SC = SC_DEFAULT if maxcnt <= 128 * SC_DEFAULT else (maxcnt + 127) // 128 + 2
    FD = C * SC

    ident = np.eye(128, dtype=np.float16)
    in_maps = []
    for n in range(N_IMG):
        ids, cnts = percore[n]
        pixgrid = np.zeros((128, FD), dtype=np.int64)  # pad -> pixel 0 (w=0)
        wgrid = np.zeros((128, FD), dtype=np.float16)
        off = 0
        for c in range(C):
            ncnt = int(cnts[c])
            seg = ids[off : off + ncnt]
            off += ncnt
            s = np.arange(ncnt)
            p, f = s % 128, c * SC + s // 128
            pixgrid[p, f] = seg
            wgrid[p, f] = wf[n][seg]
        g = xs[n][:, pixgrid.reshape(-1)]            # [C, 128*FD]
        xg = np.ascontiguousarray(
            g.reshape(C, 128, FD).transpose(1, 0, 2).reshape(128, C * FD)
        ).astype(np.float16)
        in_maps.append({"x": xg, "w": wgrid, "ident": ident})
    return in_maps, size, SC


def _combine(res_list, size) -> np.ndarray:
    S = 0.0
    for n in range(N_IMG):
        o = np.asarray(res_list[n]["out"], dtype=np.float64)
        S += o[:, 1:].sum() - o[:, 0].sum()          # S2 - S1
    loss = np.float32(-(S / size))
    return np.asarray(loss, dtype=np.float32)


def run_device(in_maps, SC=SC_DEFAULT, trace=False, **kwargs):
    from concourse.bass_utils import run_bass_kernel_spmd

    nc = _get_nc(SC)
    return run_bass_kernel_spmd(
        nc, in_maps, core_ids=list(range(N_IMG)), trace=trace, **kwargs
    )


def kernel(predict, target, confidence, accuracies, n_bin) -> np.ndarray:
    in_maps, size, SC = _prepare(predict, target, confidence, accuracies, n_bin)
    res = run_device(in_maps, SC=SC)
    return _combine(res.results, size)
